# revision 32
# baseline (speedup 1.0000x reference)
"""MultiHeadAttention Trainium2 kernel (8-core SPMD, head/batch sharded).

Reference semantics (E=1024, H=16, D=64, B=2, S=2048):
    qp = (q @ wq.T + bq).reshape(B, H, S, D)   # RAW view, not transpose!
    scores = qp @ kp^T * 1/sqrt(E); attn = softmax(scores)
    out = (attn @ vp).reshape(B, S, E) @ wo.T + bo

Because the reshape is a raw view, head h of batch b corresponds to the
contiguous 128-row block rows[128h:128h+128] of the projected [S, E]
matrix, viewed as [2048, 64].  Each core therefore only needs 512 rows of
q/k/v (4 heads) plus the full weight matrices.

Inside each head we use the permuted sequence order i' = 128r + a
(original in-head index i = 16a + r, a=row-in-block 0..127, r=col-block
0..15).  This is a symmetric permutation of Q/K/V rows, so softmax+AV
commute with it; it makes every layout matmul-native.

v3 notes (vs v2 at 212us):
  * HW model (from the v2 trace): every matmul streams ~1 moving column
    per cycle at 2.4GHz regardless of dtype; fp8 DoubleRow packs 2
    contract rows per partition (256/pass); tile_position row-packed
    pairs run CONCURRENTLY.  fp8 for AV/V/out-proj is numerically dead
    (attention weights are a near-uniform mean, so e4m3 noise doesn't
    average out) -- v3 is pure scheduling, numerics identical to v2.
  * Host pre-interleaves the fp8 DR layouts so each w/x superchunk is a
    single contiguous DMA (v2's (j o) rearrange split each into 2 chained
    instructions, delaying the first matmul to 10.5us).
  * Prefetch spread over 4 queues (sync/scalar/vector/gpsimd), first-use
    first: wq0+xq0 land ~1.5us after the queue engines boot.
  * Proj drains: vl-pairs merged into [64,2,2,128] ops, direct halves on
    ScalarE (activation Copy w/ scale), staged halves on DVE -- the v2
    ring stall (5.5us) came from 16 serial DVE drains.
  * Tails without DRAM: DVE reciprocal_approx_fast straight off the av
    PSUM denominator row, gpsimd partition_broadcast (SBUF->SBUF), and
    the normalize tensor_tensors read av PSUM directly.  No avd copies
    (16 ScalarE ops gone), no den/rec DRAM round-trips (48 dma_starts
    ~600ns engine time each, plus the Vector-FIFO head-of-line parks
    that idled the PE ~1.4us per unit boundary).
  * V/out-proj matmuls fused to m=1024; out-proj reordered mb=0..3 with
    the last unit's tail stages dripped before mb0/mb1 so the PE never
    idles at the end (v2 lost ~12us of gaps there and dropped to the
    1.2GHz clock gate for the final 22us).
  * exp split ScalarE 9 / DVE 7 per unit (DVE also runs the 2 recip
    tails; measured 1.22us vs 1.1us per 128x1024 chunk).
"""

import numpy as np

import concourse.bass as bass
import concourse.mybir as mybir
import concourse.tile as tile
from concourse import bacc
from concourse.bass_utils import run_bass_kernel_spmd

B, S, E = 2, 2048, 1024
H, D = 16, 64
HEADS_PER_CORE = 4
ROWS = 512  # rows of the [S,E] projected matrix handled per core
N_CORES = 8
SCALE = 1.0 / float(np.sqrt(np.float32(E)))

F32 = mybir.dt.float32
BF16 = mybir.dt.bfloat16
I16 = mybir.dt.int16
F8E4 = mybir.dt.float8e4
AF = mybir.ActivationFunctionType

# Q/K projections run in fp8e4m3 DoubleRow (K=256 per matmul).  Weights are
# host-scaled by W8SCALE to sit in e4m3's normal range; the drain divides out.
W8SCALE = 64.0

# IEEE bit-trick exp2 in bf16: exp(SCALE*x) ~= bits_as_bf16(AEXP*x + BEXP).
LOG2E = 1.4426950408889634
AEXP = float(2**7 * LOG2E) * SCALE
BEXP = float(2**7 * (127 - 0.0434609) + 0.5)

RUNAHEAD = 3  # QK chunks in flight ahead of AV (PSUM sc ring = 3)

DEBUG_TAPS = False  # when True, dump qT/kT/vones/oT as extra outputs


def dve_exp_chunk(u, c):
    """Strict alternation: DVE (bit-exp) odd chunks, ScalarE (true exp)
    even.  A run of same-engine chunks at the unit end serializes the
    last AVs behind one engine."""
    return (c + u) % 2 == 1


def build_nc():
    nc = bacc.Bacc(
        "TRN2",
        target_bir_lowering=False,
        debug=False,
        num_devices=N_CORES,
    )

    # DRAM parameters (per-core shapes; host passes per-core slices).
    # fp8 DR layouts are pre-interleaved on host: chunk k, partition j,
    # o in {0,1} holds contract row 256k + 2j + o.
    xq = nc.dram_tensor("xq", [4, 128, 2, ROWS], F8E4, kind="ExternalInput").ap()
    xk = nc.dram_tensor("xk", [4, 128, 2, ROWS], F8E4, kind="ExternalInput").ap()
    xv = nc.dram_tensor("xv", [E, ROWS], BF16, kind="ExternalInput").ap()
    wq = nc.dram_tensor("wq", [4, 128, 2, E], F8E4, kind="ExternalInput").ap()
    wk = nc.dram_tensor("wk", [4, 128, 2, E], F8E4, kind="ExternalInput").ap()
    bq = nc.dram_tensor("bq", [1, E], BF16, kind="ExternalInput").ap()
    bk = nc.dram_tensor("bk", [1, E], BF16, kind="ExternalInput").ap()
    wv = nc.dram_tensor("wv", [E + 1, E], BF16, kind="ExternalInput").ap()
    wo = nc.dram_tensor("wo", [E + 1, E], BF16, kind="ExternalInput").ap()
    y = nc.dram_tensor("y", [ROWS, E], F32, kind="ExternalOutput").ap()
    taps = None
    if DEBUG_TAPS:
        taps = {
            "qT_d": nc.dram_tensor("qT_d", [128, 2, 16, 128], BF16, kind="ExternalOutput").ap(),
            "kT_d": nc.dram_tensor("kT_d", [128, 2, 16, 128], BF16, kind="ExternalOutput").ap(),
            "vo_d": nc.dram_tensor("vo_d", [128, 16, 128], BF16, kind="ExternalOutput").ap(),
            "oT_d": nc.dram_tensor("oT_d", [128, 8, ROWS], BF16, kind="ExternalOutput").ap(),
        }

    with tile.TileContext(nc) as tc:
        build_tile_kernel(tc, xq, xk, xv, wq, wk, bq, bk, wv, wo, y, taps)

    nc.compile()
    return nc


def build_tile_kernel(tc, xq, xk, xv, wq, wk, bq, bk, wv, wo, y, taps=None):
    nc = tc.nc

    with (
        tc.tile_pool(name="persist", bufs=1) as persist,
        tc.tile_pool(name="expp", bufs=6) as expp,
        tc.tile_pool(name="tailp", bufs=4) as tailp,
        tc.tile_pool(name="outp", bufs=2) as outp,
        # PSUM: tag "sc" 3 x [128,2,512]f32 (6 banks) + tag "av" 2 x 1 bank
        tc.tile_pool(name="ps", bufs=3, space="PSUM") as ps,
    ):
        # ---------------- prefetch: 4 queues, first-use first ------------
        # sync queue: fp8 weights then wv
        wq_sb, wk_sb, wv_sb, wo_sb = [], [], [], []
        for k in range(4):
            t = persist.tile([128, 2, E], F8E4, tag=f"wq{k}", name=f"wq{k}")
            nc.sync.dma_start(out=t, in_=wq[k])
            wq_sb.append(t)
        # scalar queue: fp8 x then biases
        xq_sb, xk_sb, xv_sb = [], [], []
        for k in range(4):
            t = persist.tile([128, 2, ROWS], F8E4, tag=f"xq{k}", name=f"xq{k}")
            nc.scalar.dma_start(out=t, in_=xq[k])
            xq_sb.append(t)
        for k in range(4):
            t = persist.tile([128, 2, E], F8E4, tag=f"wk{k}", name=f"wk{k}")
            nc.sync.dma_start(out=t, in_=wk[k])
            wk_sb.append(t)
        for k in range(4):
            t = persist.tile([128, 2, ROWS], F8E4, tag=f"xk{k}", name=f"xk{k}")
            nc.scalar.dma_start(out=t, in_=xk[k])
            xk_sb.append(t)
        bq_row = persist.tile([1, E], BF16, tag="bq_row")
        bk_row = persist.tile([1, E], BF16, tag="bk_row")
        bv_row = persist.tile([1, E], BF16, tag="bv_row")
        bo_row = persist.tile([1, E], BF16, tag="bo_row")
        nc.scalar.dma_start(out=bq_row, in_=bq)
        nc.scalar.dma_start(out=bk_row, in_=bk)
        for k in range(8):
            t = persist.tile([128, E], BF16, tag=f"wv{k}", name=f"wv{k}")
            nc.sync.dma_start(out=t, in_=wv[128 * k : 128 * k + 128, :])
            wv_sb.append(t)
        # gpsimd queue: aug bias rows, xv, then wo
        nc.gpsimd.dma_start(out=bv_row, in_=wv[E : E + 1, :])
        nc.gpsimd.dma_start(out=bo_row, in_=wo[E : E + 1, :])
        for k in range(8):
            t = persist.tile([128, ROWS], BF16, tag=f"xv{k}", name=f"xv{k}")
            nc.gpsimd.dma_start(out=t, in_=xv[128 * k : 128 * k + 128, :])
            xv_sb.append(t)
        for k in range(8):
            t = persist.tile([128, E], BF16, tag=f"wo{k}", name=f"wo{k}")
            nc.gpsimd.dma_start(out=t, in_=wo[128 * k : 128 * k + 128, :])
            wo_sb.append(t)

        # ---------------- persistent SBUF tensors ----------------
        # qT/kT: [128, pair, r, a]; head h lives at partitions 64*(h%2)..+64,
        # pair index h//2.  Value at [64*(h%2)+d, h//2, r, a] = proj[128h+a, 64r+d].
        qT = persist.tile([128, 2, 16, 128], BF16)
        kT = persist.tile([128, 2, 16, 128], BF16)
        # vones per head: [128(a), 16(r), 65]; [...,:64] = vp rows, [...,64] = 1.0
        vones = [
            persist.tile([128, 16, D + 1], BF16, tag=f"vones{h}", name=f"vones{h}")
            for h in range(4)
        ]
        # oT: attention output, transposed for the out-projection:
        # [128(e%128), 8(e//128), 512(m)]  where e = 64r+d, m = 128h+a.
        oT = persist.tile([128, 8, ROWS], BF16)
        for h in range(4):
            nc.gpsimd.memset(vones[h][:, :, D : D + 1], 1.0)
        x_ones = persist.tile([1, ROWS], BF16, tag="x_ones")
        nc.vector.memset(x_ones, 1.0)

        # broadcast f32 biases for the V / out-proj drains
        bv_bc = persist.tile([128, E], F32, tag="bv_bc")
        bo_bc = persist.tile([128, E], F32, tag="bo_bc")
        bv_f = persist.tile([1, E], F32, tag="bv_f")
        bo_f = persist.tile([1, E], F32, tag="bo_f")
        nc.vector.tensor_copy(bv_f, bv_row)
        nc.vector.tensor_copy(bo_f, bo_row)
        nc.gpsimd.partition_broadcast(bv_bc, bv_f)
        nc.gpsimd.partition_broadcast(bo_bc, bo_f)

        # ---------------- Q / K projections (transposed layout) ----------
        # Halves of Q and K interleave so each PSUM ring slot's reuse
        # distance is a full half (~2.7us) instead of 2 matmuls -- drains
        # finish long before their bank is reallocated.
        stg_q = persist.tile([128, 8, 2, 128], BF16, tag="stg_q", name="stg_q")
        stg_k = persist.tile([128, 8, 2, 128], BF16, tag="stg_k", name="stg_k")
        proj_half(tc, ps, wq_sb, xq_sb, bq_row, x_ones, qT, stg_q, "q", 0)
        proj_half(tc, ps, wk_sb, xk_sb, bk_row, x_ones, kT, stg_k, "k", 0)
        proj_half(tc, ps, wq_sb, xq_sb, bq_row, x_ones, qT, stg_q, "q", 1)
        proj_half(tc, ps, wk_sb, xk_sb, bk_row, x_ones, kT, stg_k, "k", 1)
        for dst, stg in ((qT, stg_q), (kT, stg_k)):
            for pr in range(2):
                # staged upper half (q=1): r=2v+1 data for even-parity heads
                nc.scalar.dma_start(out=dst[0:64, pr, 1::2, :], in_=stg[64:128, :, pr, :])
                # staged lower half (q=0): r=2v data for odd-parity heads
                nc.scalar.dma_start(out=dst[64:128, pr, 0::2, :], in_=stg[0:64, :, pr, :])

        # ---------------- V projection (natural layout into vones) -------
        for h in range(4):
            acct = ps.tile([128, 2, ROWS], F32, tag="sc", name=f"accv{h}")
            for k in range(8):
                for g in range(2):
                    nc.tensor.matmul(
                        acct[:, g, :],
                        xv_sb[k][:, 128 * h : 128 * h + 128],
                        wv_sb[k][:, 512 * g : 512 * g + 512],
                        start=(k == 0),
                        stop=(k == 7),
                    )
            nc.vector.tensor_tensor(
                out=vones[h][:, :, 0:D],
                in0=acct.rearrange("p g (rr d) -> p (g rr) d", d=D),
                in1=bv_bc.rearrange("p (r d) -> p r d", d=D),
                op=mybir.AluOpType.add,
            )

        # ---------------- attention: 8 units of (head pair, 512 queries) --
        # Tail stages are deferred into a FIFO and dripped one-per-chunk into
        # the NEXT unit so no stage parks an engine FIFO mid-attention.
        tail_stages = []
        for pr in range(2):
            for iq in range(4):
                attention_unit(
                    tc, ps, expp, tailp, qT, kT, vones, oT, pr, iq, tail_stages
                )

        # ---------------- output projection ----------------
        # mb = head block.  The last unit (heads 2/3, iq=3) still has its
        # tail stages pending; emit half 0's chain before mb0 and the rest
        # before mb1 so they complete on Scalar/Vector/GpSimd while the PE
        # runs the (independent) head-0/1 blocks.
        for mb in range(4):
            if mb == 0:
                while len(tail_stages) > 8:
                    tail_stages.pop(0)()
            elif mb == 1:
                while tail_stages:
                    tail_stages.pop(0)()
            acct = ps.tile([128, 2, ROWS], F32, tag="sc", name=f"acco{mb}")
            for v in range(8):
                for g in range(2):
                    nc.tensor.matmul(
                        acct[:, g, :],
                        oT[:, v, 128 * mb : 128 * mb + 128],
                        wo_sb[v][:, 512 * g : 512 * g + 512],
                        start=(v == 0),
                        stop=(v == 7),
                    )
            y_sb = outp.tile([128, E], F32, tag="ysb", name=f"ysb{mb}")
            nc.vector.tensor_tensor(
                out=y_sb,
                in0=acct.rearrange("p g m -> p (g m)"),
                in1=bo_bc,
                op=mybir.AluOpType.add,
            )
            nc.scalar.dma_start(out=y[128 * mb : 128 * mb + 128, :], in_=y_sb)

        if taps is not None:
            nc.scalar.dma_start(out=taps["qT_d"], in_=qT)
            nc.scalar.dma_start(out=taps["kT_d"], in_=kT)
            nc.scalar.dma_start(out=taps["vo_d"], in_=vones[0])
            nc.scalar.dma_start(out=taps["oT_d"], in_=oT)


def proj_half(tc, ps, w_sb, x_sb, bias_row, x_ones, dst, stg, nm, half):
    """One feature-half (v = 4*half + 0..3) of a projection x @ w.T into the
    per-head transposed layout `dst`.

    Feature-block v of the PSUM output holds features n = 128v + 64p + d at
    partition 64p + d (p = upper/lower half), i.e. r = 2v + p.  Head h wants
    its data at partition half h%2, so blocks with p == h%2 copy straight
    through and the other half bounce via a staging tile and two
    partition-shifting SBUF->SBUF DMAs (emitted by the caller).  Bias rides
    a 9th K=1 matmul against a memset ones row.  Per PSUM tile the drains
    are merged (vl pairs) and split across ScalarE (direct) / DVE (staged),
    emitted right after that tile's stop matmul.
    """
    nc = tc.nc
    acct = [
        ps.tile([128, 2, ROWS], F32, tag="sc", name=f"acc{nm}{half}{t}")
        for t in range(2)
    ]
    accs = [acct[0][:, 0, :], acct[0][:, 1, :], acct[1][:, 0, :], acct[1][:, 1, :]]
    for k in range(4):  # k-outer so compute streams behind the w DMA
        for vl in range(4):
            v = 4 * half + vl
            nc.tensor.matmul(
                accs[vl],
                w_sb[k][:, :, 128 * v : 128 * v + 128],
                x_sb[k],
                start=(k == 0),
                stop=False,
                perf_mode=mybir.MatmulPerfMode.DoubleRow,
            )

    def drain_tile(t):
        v0 = 4 * half + 2 * t
        # src m-dim split: m = 256*pr + 128*par + a (head hl = 2*pr+par)
        src = acct[t].rearrange("d vl (pr par a) -> d par pr vl a", par=2, a=128)
        for p in range(2):
            # direct: heads with h%2 == p whose data sits in psum half p
            # -> ScalarE copy (divides out the host-side W8SCALE)
            nc.scalar.activation(
                out=dst[64 * p : 64 * p + 64, :, 2 * v0 + p : 2 * v0 + p + 3 : 2, :],
                in_=src[64 * p : 64 * p + 64, p],
                func=AF.Copy,
                scale=1.0 / W8SCALE,
            )
            # staged: psum half q holds r=2v+q data; heads of the OTHER
            # parity p=1-q need it partition-shifted.  Stage on DVE,
            # shift later via DMA.
            q = 1 - p
            nc.vector.tensor_scalar(
                out=stg[64 * q : 64 * q + 64, v0 : v0 + 2, :, :].rearrange(
                    "d v pr a -> d pr v a"
                ),
                in0=src[64 * q : 64 * q + 64, p],
                scalar1=1.0 / W8SCALE,
                scalar2=None,
                op0=mybir.AluOpType.mult,
            )

    for t in range(2):
        for vl in (2 * t, 2 * t + 1):
            v = 4 * half + vl
            nc.tensor.matmul(
                accs[vl],
                bias_row[:, 128 * v : 128 * v + 128],
                x_ones,
                start=False,
                stop=True,
            )
        drain_tile(t)


def attention_unit(tc, ps, expp, tailp, qT, kT, vones, oT, pr, iq, tail_stages):
    """Heads (2pr, 2pr+1) x queries i' in [512*iq, 512*iq+512).

    Per key-chunk c (128 keys): QK pair (row-strip packed, concurrent),
    one whole-chunk exp on ScalarE or VectorE (alternating), AV pair into
    the per-half [65, 512] accumulators (row 64 = softmax denominator via
    the vones ones-column).  QK runs RUNAHEAD chunks ahead of AV so the PE
    never waits on exp; the sc ring (3 tiles) makes that legal.

    Tail (per half): DVE reciprocal straight off the av PSUM denominator
    row, gpsimd partition_broadcast, then the normalize tensor_tensors
    read av PSUM directly -- no SBUF drain copy, no DRAM bounce.  The av
    pool (bufs=2) keeps the PSUM alive until the normalizes (dripped into
    the next unit) complete.
    """
    nc = tc.nc
    u = 4 * pr + iq
    qslice = slice(4 * iq, 4 * iq + 4)  # r-blocks of this query window

    av = [
        ps.tile([D + 1, ROWS], F32, tag="av", name=f"av{u}{half}", bufs=2)
        for half in range(2)
    ]
    ex = {}

    def emit_qk_exp(c):
        sc = ps.tile([128, 2, ROWS], F32, tag="sc", name=f"sc{u}{c}")
        for half in range(2):
            base = 64 * half
            nc.tensor.matmul(
                sc[:, half, :],
                kT[base : base + 64, pr, c, :],
                qT[base : base + 64, pr, qslice, :],
                start=True,
                stop=True,
                tile_position=(base, 0),
            )
        sc_flat = sc.rearrange("p h m -> p (h m)")
        if not dve_exp_chunk(u, c):
            e = expp.tile([128, 2, ROWS], BF16, tag="ex", name=f"ex{u}{c}")
            nc.scalar.activation(
                e.rearrange("p h m -> p (h m)"), sc_flat, AF.Exp, scale=SCALE
            )
        else:
            ei = expp.tile([128, 2, ROWS], I16, tag="ex", name=f"exi{u}{c}")
            nc.vector.tensor_scalar(
                out=ei.rearrange("p h m -> p (h m)"),
                in0=sc_flat,
                scalar1=AEXP,
                scalar2=BEXP,
                op0=mybir.AluOpType.mult,
                op1=mybir.AluOpType.add,
            )
            e = ei.bitcast(BF16)
        ex[c] = e

    def emit_av(c):
        for half in range(2):
            h = 2 * pr + half
            nc.tensor.matmul(
                av[half],
                vones[h][:, c, :],
                ex[c][:, half, :],
                start=(c == 0),
                stop=(c == 15),
            )
        del ex[c]

    for c in range(16):
        emit_qk_exp(c)
        if c >= RUNAHEAD:
            emit_av(c - RUNAHEAD)
        # 16 tail stages per unit over ~15 chunk slots: drip a second one
        # in the back half so the queue fully drains every unit
        if c >= 1 and tail_stages:
            tail_stages.pop(0)()
        if c >= 8 and tail_stages:
            tail_stages.pop(0)()
    for c in range(16 - RUNAHEAD, 16):
        emit_av(c)

    # Tail stages for this unit, dripped into the next unit (or the output
    # projection for the last one).  8 stages per half: one ScalarE copy
    # drains av PSUM to SBUF (GPSIMD cannot read PSUM), the denominator row
    # bounces [1,512]->[4,128] on the free sync queue, DVE reciprocals it
    # in ~270ns, a second tiny DMA merges back to [1,512], and the (mostly
    # idle) GpSimd broadcasts + runs the normalize mults.
    for half in range(2):
        h = 2 * pr + half
        avh = av[half]
        avd = tailp.tile([D + 1, ROWS], F32, tag="avd", name=f"avd{u}{half}")
        den_t = tailp.tile([4, 128], F32, tag="dent", name=f"dent{u}{half}")
        rec_row = tailp.tile([1, ROWS], F32, tag="recr", name=f"recr{u}{half}")
        rec_bc = tailp.tile([64, ROWS], F32, tag="recbc", name=f"recbc{u}{half}")
        stg_o = tailp.tile([64, 2, 128], BF16, tag="stgo", name=f"stgo{u}{half}")

        def stage_avd(avh=avh, avd=avd):
            nc.scalar.copy(avd, avh)

        def stage_dendma(avd=avd, den_t=den_t):
            nc.sync.dma_start(out=den_t, in_=avd[D : D + 1, :])

        def stage_recip(den_t=den_t):
            nc.vector.reciprocal_approx_fast(out=den_t, in_=den_t)

        def stage_recdma(den_t=den_t, rec_row=rec_row):
            nc.sync.dma_start(out=rec_row, in_=den_t)

        def stage_bcast(rec_row=rec_row, rec_bc=rec_bc):
            nc.gpsimd.partition_broadcast(rec_bc, rec_row)

        def stage_norm_even(avd=avd, rec_bc=rec_bc, h=h):
            av_r = avd[0:D].rearrange("d (rl a) -> d rl a", a=128)
            bc_r = rec_bc.rearrange("d (rl a) -> d rl a", a=128)
            # even rl (r = 4iq+rl even): partitions already correct (e%128 = d)
            nc.gpsimd.tensor_tensor(
                out=oT[0:64, 2 * iq : 2 * iq + 2, 128 * h : 128 * h + 128],
                in0=av_r[:, 0::2, :],
                in1=bc_r[:, 0::2, :],
                op=mybir.AluOpType.mult,
            )

        def stage_norm_odd(avd=avd, rec_bc=rec_bc, stg_o=stg_o):
            av_r = avd[0:D].rearrange("d (rl a) -> d rl a", a=128)
            bc_r = rec_bc.rearrange("d (rl a) -> d rl a", a=128)
            nc.gpsimd.tensor_tensor(
                out=stg_o,
                in0=av_r[:, 1::2, :],
                in1=bc_r[:, 1::2, :],
                op=mybir.AluOpType.mult,
            )

        def stage_stgdma(stg_o=stg_o, h=h):
            # odd rl: partition-shift DMA to oT[64:], on the idle sync queue
            nc.sync.dma_start(
                out=oT[64:128, 2 * iq : 2 * iq + 2, 128 * h : 128 * h + 128],
                in_=stg_o,
            )

        tail_stages.append(stage_avd)
        tail_stages.append(stage_dendma)
        tail_stages.append(stage_recip)
        tail_stages.append(stage_recdma)
        tail_stages.append(stage_bcast)
        tail_stages.append(stage_norm_even)
        tail_stages.append(stage_norm_odd)
        tail_stages.append(stage_stgdma)


_NC_CACHE = {}


def get_nc():
    if "nc" not in _NC_CACHE:
        _NC_CACHE["nc"] = build_nc()
    return _NC_CACHE["nc"]


def shard_inputs(q, k, v, wq, bq, wk, bk, wv, bv, wo, bo):
    """Build the 8 per-core input maps (host-side transposes/augments)."""

    import ml_dtypes

    bf16 = ml_dtypes.bfloat16
    f8 = ml_dtypes.float8_e4m3fn

    def aug_w(w, b):
        return np.concatenate(
            [np.ascontiguousarray(np.asarray(w, np.float32).T),
             np.asarray(b, np.float32)[None, :]],
            axis=0,
        ).astype(bf16)

    def w8_il(w):
        # [E, E] -> [4, 128, 2, E]: chunk k, partition j, o holds contract
        # row 256k + 2j + o of w.T (pre-interleaved DoubleRow layout)
        wt = np.ascontiguousarray(
            np.asarray(w, np.float32).T * np.float32(W8SCALE)
        ).astype(f8)
        return np.ascontiguousarray(wt.reshape(4, 128, 2, E))

    wq_a, wk_a = w8_il(wq), w8_il(wk)
    bq_a = (np.asarray(bq, np.float32) * np.float32(W8SCALE))[None, :].astype(bf16)
    bk_a = (np.asarray(bk, np.float32) * np.float32(W8SCALE))[None, :].astype(bf16)
    wv_a, wo_a = aug_w(wv, bv), aug_w(wo, bo)

    in_maps = []
    for c in range(N_CORES):
        b = c // 4
        r0 = 512 * (c % 4)
        sl = slice(r0, r0 + ROWS)

        def t_x(x, dt):
            return np.ascontiguousarray(np.asarray(x[b, sl, :], np.float32).T).astype(
                dt
            )

        def x8_il(x):
            return np.ascontiguousarray(t_x(x, f8).reshape(4, 128, 2, ROWS))

        in_maps.append(
            {
                "xq": x8_il(q),
                "xk": x8_il(k),
                "xv": t_x(v, bf16),
                "wq": wq_a,
                "wk": wk_a,
                "bq": bq_a,
                "bk": bk_a,
                "wv": wv_a,
                "wo": wo_a,
            }
        )
    return in_maps


def assemble_output(results):
    out = np.empty((B, S, E), np.float32)
    for c in range(N_CORES):
        b = c // 4
        r0 = 512 * (c % 4)
        out[b, r0 : r0 + ROWS, :] = results[c]["y"]
    return out


def kernel(q, k, v, wq, bq, wk, bk, wv, bv, wo, bo, **run_kwargs):
    nc = get_nc()
    in_maps = shard_inputs(q, k, v, wq, bq, wk, bk, wv, bv, wo, bo)
    res = run_bass_kernel_spmd(nc, in_maps, list(range(N_CORES)), **run_kwargs)
    out = assemble_output(res.results)
    if run_kwargs:
        return out, res
    return out


# revision 34
# speedup vs baseline: 1.6213x; 1.6213x over previous
"""MultiHeadAttention Trainium2 kernel (8-core SPMD, head/batch sharded).

Reference semantics (E=1024, H=16, D=64, B=2, S=2048):
    qp = (q @ wq.T + bq).reshape(B, H, S, D)   # RAW view, not transpose!
    scores = qp @ kp^T * 1/sqrt(E); attn = softmax(scores)
    out = (attn @ vp).reshape(B, S, E) @ wo.T + bo

Because the reshape is a raw view, head h of batch b corresponds to the
contiguous 128-row block rows[128h:128h+128] of the projected [S, E]
matrix, viewed as [2048, 64].  Each core therefore only needs 512 rows of
q/k/v (4 heads) plus the full weight matrices.

Inside each head we use the permuted sequence order i' = 128r + a
(original in-head index i = 16a + r, a=row-in-block 0..127, r=col-block
0..15).  This is a symmetric permutation of Q/K/V rows, so softmax+AV
commute with it; it makes every layout matmul-native.

v3 notes (vs v2 at 212us):
  * HW model (from the v2 trace): every matmul streams ~1 moving column
    per cycle at 2.4GHz regardless of dtype; fp8 DoubleRow packs 2
    contract rows per partition (256/pass); tile_position row-packed
    pairs run CONCURRENTLY.  fp8 for AV/V/out-proj is numerically dead
    (attention weights are a near-uniform mean, so e4m3 noise doesn't
    average out) -- v3 is pure scheduling, numerics identical to v2.
  * Host pre-interleaves the fp8 DR layouts so each w/x superchunk is a
    single contiguous DMA (v2's (j o) rearrange split each into 2 chained
    instructions, delaying the first matmul to 10.5us).
  * Prefetch spread over 4 queues (sync/scalar/vector/gpsimd), first-use
    first: wq0+xq0 land ~1.5us after the queue engines boot.
  * Proj drains: vl-pairs merged into [64,2,2,128] ops, direct halves on
    ScalarE (activation Copy w/ scale), staged halves on DVE -- the v2
    ring stall (5.5us) came from 16 serial DVE drains.
  * Tails without DRAM: DVE reciprocal_approx_fast straight off the av
    PSUM denominator row, gpsimd partition_broadcast (SBUF->SBUF), and
    the normalize tensor_tensors read av PSUM directly.  No avd copies
    (16 ScalarE ops gone), no den/rec DRAM round-trips (48 dma_starts
    ~600ns engine time each, plus the Vector-FIFO head-of-line parks
    that idled the PE ~1.4us per unit boundary).
  * V/out-proj matmuls fused to m=1024; out-proj reordered mb=0..3 with
    the last unit's tail stages dripped before mb0/mb1 so the PE never
    idles at the end (v2 lost ~12us of gaps there and dropped to the
    1.2GHz clock gate for the final 22us).
  * exp split ScalarE 9 / DVE 7 per unit (DVE also runs the 2 recip
    tails; measured 1.22us vs 1.1us per 128x1024 chunk).
"""

import numpy as np

import concourse.bass as bass
import concourse.mybir as mybir
import concourse.tile as tile
from concourse import bacc
from concourse.bass_utils import run_bass_kernel_spmd

B, S, E = 2, 2048, 1024
H, D = 16, 64
HEADS_PER_CORE = 4
ROWS = 512  # rows of the [S,E] projected matrix handled per core
N_CORES = 8
SCALE = 1.0 / float(np.sqrt(np.float32(E)))

F32 = mybir.dt.float32
BF16 = mybir.dt.bfloat16
I16 = mybir.dt.int16
F8E4 = mybir.dt.float8e4
AF = mybir.ActivationFunctionType

# Q/K projections run in fp8e4m3 DoubleRow (K=256 per matmul).  Weights are
# host-scaled by W8SCALE to sit in e4m3's normal range; the drain divides out.
W8SCALE = 64.0

# IEEE bit-trick exp2 in bf16: exp(SCALE*x) ~= bits_as_bf16(AEXP*x + BEXP).
LOG2E = 1.4426950408889634
AEXP = float(2**7 * LOG2E) * SCALE
BEXP = float(2**7 * (127 - 0.0434609) + 0.5)

RUNAHEAD = 3  # QK chunks in flight ahead of AV (PSUM sc ring = 3)

DEBUG_TAPS = False  # when True, dump qT/kT/vones/oT as extra outputs


def dve_exp_chunk(u, c):
    """Strict alternation: DVE (bit-exp) odd chunks, ScalarE (true exp)
    even.  A run of same-engine chunks at the unit end serializes the
    last AVs behind one engine."""
    return (c + u) % 2 == 1


def build_nc():
    nc = bacc.Bacc(
        "TRN2",
        target_bir_lowering=False,
        debug=False,
        num_devices=N_CORES,
    )

    # DRAM parameters (per-core shapes; host passes per-core slices).
    # fp8 DR layouts are pre-interleaved on host: chunk k, partition j,
    # o in {0,1} holds contract row 256k + 2j + o.
    xq = nc.dram_tensor("xq", [4, 128, 2, ROWS], F8E4, kind="ExternalInput").ap()
    xk = nc.dram_tensor("xk", [4, 128, 2, ROWS], F8E4, kind="ExternalInput").ap()
    xv = nc.dram_tensor("xv", [E, ROWS], BF16, kind="ExternalInput").ap()
    wq = nc.dram_tensor("wq", [4, 128, 2, E], F8E4, kind="ExternalInput").ap()
    wk = nc.dram_tensor("wk", [4, 128, 2, E], F8E4, kind="ExternalInput").ap()
    bq = nc.dram_tensor("bq", [1, E], BF16, kind="ExternalInput").ap()
    bk = nc.dram_tensor("bk", [1, E], BF16, kind="ExternalInput").ap()
    wv = nc.dram_tensor("wv", [E + 1, E], BF16, kind="ExternalInput").ap()
    wo = nc.dram_tensor("wo", [E + 1, E], BF16, kind="ExternalInput").ap()
    y = nc.dram_tensor("y", [ROWS, E], F32, kind="ExternalOutput").ap()
    taps = None
    if DEBUG_TAPS:
        taps = {
            "qT_d": nc.dram_tensor("qT_d", [128, 2, 16, 128], BF16, kind="ExternalOutput").ap(),
            "kT_d": nc.dram_tensor("kT_d", [128, 2, 16, 128], BF16, kind="ExternalOutput").ap(),
            "vo_d": nc.dram_tensor("vo_d", [128, 16, 128], BF16, kind="ExternalOutput").ap(),
            "oT_d": nc.dram_tensor("oT_d", [128, 8, ROWS], BF16, kind="ExternalOutput").ap(),
        }

    with tile.TileContext(nc) as tc:
        build_tile_kernel(tc, xq, xk, xv, wq, wk, bq, bk, wv, wo, y, taps)

    nc.compile()
    return nc


def build_tile_kernel(tc, xq, xk, xv, wq, wk, bq, bk, wv, wo, y, taps=None):
    nc = tc.nc

    with (
        tc.tile_pool(name="persist", bufs=1) as persist,
        tc.tile_pool(name="expp", bufs=6) as expp,
        tc.tile_pool(name="tailp", bufs=4) as tailp,
        tc.tile_pool(name="outp", bufs=2) as outp,
        tc.tile_pool(name="dramp", bufs=4, space="DRAM") as dramp,
        # PSUM: tag "sc" 3 x [128,2,512]f32 (6 banks) + tag "av" 2 x 1 bank
        tc.tile_pool(name="ps", bufs=3, space="PSUM") as ps,
    ):
        # ---------------- prefetch: 4 queues, first-use first ------------
        # sync queue: fp8 weights then wv
        wq_sb, wk_sb, wv_sb, wo_sb = [], [], [], []
        for k in range(4):
            t = persist.tile([128, 2, E], F8E4, tag=f"wq{k}", name=f"wq{k}")
            nc.sync.dma_start(out=t, in_=wq[k])
            wq_sb.append(t)
        # scalar queue: fp8 x then biases
        xq_sb, xk_sb, xv_sb = [], [], []
        for k in range(4):
            t = persist.tile([128, 2, ROWS], F8E4, tag=f"xq{k}", name=f"xq{k}")
            nc.scalar.dma_start(out=t, in_=xq[k])
            xq_sb.append(t)
        for k in range(4):
            t = persist.tile([128, 2, E], F8E4, tag=f"wk{k}", name=f"wk{k}")
            nc.sync.dma_start(out=t, in_=wk[k])
            wk_sb.append(t)
        for k in range(4):
            t = persist.tile([128, 2, ROWS], F8E4, tag=f"xk{k}", name=f"xk{k}")
            nc.scalar.dma_start(out=t, in_=xk[k])
            xk_sb.append(t)
        bq_row = persist.tile([1, E], BF16, tag="bq_row")
        bk_row = persist.tile([1, E], BF16, tag="bk_row")
        bv_row = persist.tile([1, E], BF16, tag="bv_row")
        bo_row = persist.tile([1, E], BF16, tag="bo_row")
        nc.scalar.dma_start(out=bq_row, in_=bq)
        nc.scalar.dma_start(out=bk_row, in_=bk)
        for k in range(8):
            t = persist.tile([128, E], BF16, tag=f"wv{k}", name=f"wv{k}")
            nc.sync.dma_start(out=t, in_=wv[128 * k : 128 * k + 128, :])
            wv_sb.append(t)
        # gpsimd queue: aug bias rows, xv, then wo
        nc.gpsimd.dma_start(out=bv_row, in_=wv[E : E + 1, :])
        nc.gpsimd.dma_start(out=bo_row, in_=wo[E : E + 1, :])
        for k in range(8):
            t = persist.tile([128, ROWS], BF16, tag=f"xv{k}", name=f"xv{k}")
            nc.gpsimd.dma_start(out=t, in_=xv[128 * k : 128 * k + 128, :])
            xv_sb.append(t)
        for k in range(8):
            t = persist.tile([128, E], BF16, tag=f"wo{k}", name=f"wo{k}")
            nc.gpsimd.dma_start(out=t, in_=wo[128 * k : 128 * k + 128, :])
            wo_sb.append(t)

        # ---------------- persistent SBUF tensors ----------------
        # qT/kT: [128, pair, r, a]; head h lives at partitions 64*(h%2)..+64,
        # pair index h//2.  Value at [64*(h%2)+d, h//2, r, a] = proj[128h+a, 64r+d].
        qT = persist.tile([128, 2, 16, 128], BF16)
        kT = persist.tile([128, 2, 16, 128], BF16)
        # vones per head: [128(a), 16(r), 65]; [...,:64] = vp rows, [...,64] = 1.0
        vones = [
            persist.tile([128, 16, D + 1], BF16, tag=f"vones{h}", name=f"vones{h}")
            for h in range(4)
        ]
        # oT: attention output, transposed for the out-projection:
        # [128(e%128), 8(e//128), 512(m)]  where e = 64r+d, m = 128h+a.
        oT = persist.tile([128, 8, ROWS], BF16)
        for h in range(4):
            nc.gpsimd.memset(vones[h][:, :, D : D + 1], 1.0)
        x_ones = persist.tile([1, ROWS], BF16, tag="x_ones")
        nc.vector.memset(x_ones, 1.0)

        # broadcast f32 biases for the V / out-proj drains
        bv_bc = persist.tile([128, E], F32, tag="bv_bc")
        bo_bc = persist.tile([128, E], F32, tag="bo_bc")
        bv_f = persist.tile([1, E], F32, tag="bv_f")
        bo_f = persist.tile([1, E], F32, tag="bo_f")
        nc.vector.tensor_copy(bv_f, bv_row)
        nc.vector.tensor_copy(bo_f, bo_row)
        nc.gpsimd.partition_broadcast(bv_bc, bv_f)
        nc.gpsimd.partition_broadcast(bo_bc, bo_f)

        # ---------------- Q / K projections (transposed layout) ----------
        # Halves of Q and K interleave so each PSUM ring slot's reuse
        # distance is a full half (~2.7us) instead of 2 matmuls -- drains
        # finish long before their bank is reallocated.
        stg_q = persist.tile([128, 8, 2, 128], BF16, tag="stg_q", name="stg_q")
        stg_k = persist.tile([128, 8, 2, 128], BF16, tag="stg_k", name="stg_k")
        proj_half(tc, ps, wq_sb, xq_sb, bq_row, x_ones, qT, stg_q, "q", 0)
        proj_half(tc, ps, wk_sb, xk_sb, bk_row, x_ones, kT, stg_k, "k", 0)
        proj_half(tc, ps, wq_sb, xq_sb, bq_row, x_ones, qT, stg_q, "q", 1)
        proj_half(tc, ps, wk_sb, xk_sb, bk_row, x_ones, kT, stg_k, "k", 1)
        for dst, stg in ((qT, stg_q), (kT, stg_k)):
            for pr in range(2):
                # staged upper half (q=1): r=2v+1 data for even-parity heads
                nc.scalar.dma_start(out=dst[0:64, pr, 1::2, :], in_=stg[64:128, :, pr, :])
                # staged lower half (q=0): r=2v data for odd-parity heads
                nc.scalar.dma_start(out=dst[64:128, pr, 0::2, :], in_=stg[0:64, :, pr, :])

        # ---------------- V projection (natural layout into vones) -------
        for h in range(4):
            acct = ps.tile([128, 2, ROWS], F32, tag="sc", name=f"accv{h}")
            for k in range(8):
                for g in range(2):
                    nc.tensor.matmul(
                        acct[:, g, :],
                        xv_sb[k][:, 128 * h : 128 * h + 128],
                        wv_sb[k][:, 512 * g : 512 * g + 512],
                        start=(k == 0),
                        stop=(k == 7),
                    )
            nc.vector.tensor_tensor(
                out=vones[h][:, :, 0:D],
                in0=acct.rearrange("p g (rr d) -> p (g rr) d", d=D),
                in1=bv_bc.rearrange("p (r d) -> p r d", d=D),
                op=mybir.AluOpType.add,
            )

        # ---------------- attention: 8 units of (head pair, 512 queries) --
        # Tail stages are deferred into a FIFO and dripped one-per-chunk into
        # the NEXT unit so no stage parks an engine FIFO mid-attention.
        tail_stages = []
        for pr in range(2):
            for iq in range(4):
                attention_unit(
                    tc, ps, expp, tailp, dramp, qT, kT, vones, oT, pr, iq,
                    tail_stages,
                )

        # ---------------- output projection ----------------
        # mb = head block.  The last unit (heads 2/3, iq=3) still has its
        # tail stages pending; emit half 0's chain before mb0 and the rest
        # before mb1 so they complete on Scalar/Vector/GpSimd while the PE
        # runs the (independent) head-0/1 blocks.
        for mb in range(4):
            if mb == 0:
                while len(tail_stages) > 8:
                    tail_stages.pop(0)()
            elif mb == 1:
                while tail_stages:
                    tail_stages.pop(0)()
            acct = ps.tile([128, 2, ROWS], F32, tag="sc", name=f"acco{mb}")
            for v in range(8):
                for g in range(2):
                    nc.tensor.matmul(
                        acct[:, g, :],
                        oT[:, v, 128 * mb : 128 * mb + 128],
                        wo_sb[v][:, 512 * g : 512 * g + 512],
                        start=(v == 0),
                        stop=(v == 7),
                    )
            y_sb = outp.tile([128, E], F32, tag="ysb", name=f"ysb{mb}")
            nc.vector.tensor_tensor(
                out=y_sb,
                in0=acct.rearrange("p g m -> p (g m)"),
                in1=bo_bc,
                op=mybir.AluOpType.add,
            )
            nc.scalar.dma_start(out=y[128 * mb : 128 * mb + 128, :], in_=y_sb)

        if taps is not None:
            nc.scalar.dma_start(out=taps["qT_d"], in_=qT)
            nc.scalar.dma_start(out=taps["kT_d"], in_=kT)
            nc.scalar.dma_start(out=taps["vo_d"], in_=vones[0])
            nc.scalar.dma_start(out=taps["oT_d"], in_=oT)


def proj_half(tc, ps, w_sb, x_sb, bias_row, x_ones, dst, stg, nm, half):
    """One feature-half (v = 4*half + 0..3) of a projection x @ w.T into the
    per-head transposed layout `dst`.

    Feature-block v of the PSUM output holds features n = 128v + 64p + d at
    partition 64p + d (p = upper/lower half), i.e. r = 2v + p.  Head h wants
    its data at partition half h%2, so blocks with p == h%2 copy straight
    through and the other half bounce via a staging tile and two
    partition-shifting SBUF->SBUF DMAs (emitted by the caller).  Bias rides
    a 9th K=1 matmul against a memset ones row.  Per PSUM tile the drains
    are merged (vl pairs) and split across ScalarE (direct) / DVE (staged),
    emitted right after that tile's stop matmul.
    """
    nc = tc.nc
    acct = [
        ps.tile([128, 2, ROWS], F32, tag="sc", name=f"acc{nm}{half}{t}")
        for t in range(2)
    ]
    accs = [acct[0][:, 0, :], acct[0][:, 1, :], acct[1][:, 0, :], acct[1][:, 1, :]]
    for k in range(4):  # k-outer so compute streams behind the w DMA
        for vl in range(4):
            v = 4 * half + vl
            nc.tensor.matmul(
                accs[vl],
                w_sb[k][:, :, 128 * v : 128 * v + 128],
                x_sb[k],
                start=(k == 0),
                stop=False,
                perf_mode=mybir.MatmulPerfMode.DoubleRow,
            )

    def drain_tile(t):
        v0 = 4 * half + 2 * t
        # src m-dim split: m = 256*pr + 128*par + a (head hl = 2*pr+par)
        src = acct[t].rearrange("d vl (pr par a) -> d par pr vl a", par=2, a=128)
        for p in range(2):
            # direct: heads with h%2 == p whose data sits in psum half p
            # -> ScalarE copy (divides out the host-side W8SCALE)
            nc.scalar.activation(
                out=dst[64 * p : 64 * p + 64, :, 2 * v0 + p : 2 * v0 + p + 3 : 2, :],
                in_=src[64 * p : 64 * p + 64, p],
                func=AF.Copy,
                scale=1.0 / W8SCALE,
            )
            # staged: psum half q holds r=2v+q data; heads of the OTHER
            # parity p=1-q need it partition-shifted.  Stage on DVE,
            # shift later via DMA.
            q = 1 - p
            nc.vector.tensor_scalar(
                out=stg[64 * q : 64 * q + 64, v0 : v0 + 2, :, :].rearrange(
                    "d v pr a -> d pr v a"
                ),
                in0=src[64 * q : 64 * q + 64, p],
                scalar1=1.0 / W8SCALE,
                scalar2=None,
                op0=mybir.AluOpType.mult,
            )

    for t in range(2):
        for vl in (2 * t, 2 * t + 1):
            v = 4 * half + vl
            nc.tensor.matmul(
                accs[vl],
                bias_row[:, 128 * v : 128 * v + 128],
                x_ones,
                start=False,
                stop=True,
            )
        drain_tile(t)


def attention_unit(tc, ps, expp, tailp, dramp, qT, kT, vones, oT, pr, iq, tail_stages):
    """Heads (2pr, 2pr+1) x queries i' in [512*iq, 512*iq+512).

    Per key-chunk c (128 keys): QK pair (row-strip packed, concurrent),
    one whole-chunk exp on ScalarE or VectorE (alternating), AV pair into
    the per-half [65, 512] accumulators (row 64 = softmax denominator via
    the vones ones-column).  QK runs RUNAHEAD chunks ahead of AV so the PE
    never waits on exp; the sc ring (3 tiles) makes that legal.

    Tail (per half): DVE reciprocal straight off the av PSUM denominator
    row, gpsimd partition_broadcast, then the normalize tensor_tensors
    read av PSUM directly -- no SBUF drain copy, no DRAM bounce.  The av
    pool (bufs=2) keeps the PSUM alive until the normalizes (dripped into
    the next unit) complete.
    """
    nc = tc.nc
    u = 4 * pr + iq
    qslice = slice(4 * iq, 4 * iq + 4)  # r-blocks of this query window

    av = [
        ps.tile([D + 1, ROWS], F32, tag="av", name=f"av{u}{half}", bufs=2)
        for half in range(2)
    ]
    ex = {}

    def emit_qk_exp(c):
        sc = ps.tile([128, 2, ROWS], F32, tag="sc", name=f"sc{u}{c}")
        for half in range(2):
            base = 64 * half
            nc.tensor.matmul(
                sc[:, half, :],
                kT[base : base + 64, pr, c, :],
                qT[base : base + 64, pr, qslice, :],
                start=True,
                stop=True,
                tile_position=(base, 0),
            )
        sc_flat = sc.rearrange("p h m -> p (h m)")
        if not dve_exp_chunk(u, c):
            e = expp.tile([128, 2, ROWS], BF16, tag="ex", name=f"ex{u}{c}")
            nc.scalar.activation(
                e.rearrange("p h m -> p (h m)"), sc_flat, AF.Exp, scale=SCALE
            )
        else:
            ei = expp.tile([128, 2, ROWS], I16, tag="ex", name=f"exi{u}{c}")
            nc.vector.tensor_scalar(
                out=ei.rearrange("p h m -> p (h m)"),
                in0=sc_flat,
                scalar1=AEXP,
                scalar2=BEXP,
                op0=mybir.AluOpType.mult,
                op1=mybir.AluOpType.add,
            )
            e = ei.bitcast(BF16)
        ex[c] = e

    def emit_av(c):
        for half in range(2):
            h = 2 * pr + half
            nc.tensor.matmul(
                av[half],
                vones[h][:, c, :],
                ex[c][:, half, :],
                start=(c == 0),
                stop=(c == 15),
            )
        del ex[c]

    for c in range(16):
        emit_qk_exp(c)
        if c >= RUNAHEAD:
            emit_av(c - RUNAHEAD)
        # 16 tail stages per unit over ~15 chunk slots: drip a second one
        # in the back half so the queue fully drains every unit
        if c >= 1 and tail_stages:
            tail_stages.pop(0)()
        if c >= 8 and tail_stages:
            tail_stages.pop(0)()
    for c in range(16 - RUNAHEAD, 16):
        emit_av(c)

    # Tail stages for this unit, dripped into the next unit (or the output
    # projection for the last one).  8 stages per half: one ScalarE copy
    # drains av PSUM to SBUF (GPSIMD cannot read PSUM), the denominator row
    # bounces [1,512]->[4,128] on the free sync queue, DVE reciprocals it
    # in ~270ns, a second tiny DMA merges back to [1,512], and the (mostly
    # idle) GpSimd broadcasts + runs the normalize mults.
    for half in range(2):
        h = 2 * pr + half
        avh = av[half]
        avd = tailp.tile([D + 1, ROWS], F32, tag="avd", name=f"avd{u}{half}")
        den_t = tailp.tile([4, 128], F32, tag="dent", name=f"dent{u}{half}")
        rec_d = dramp.tile([1, ROWS], F32, tag="recd", name=f"recd{u}{half}")
        rec_bc = tailp.tile([64, ROWS], F32, tag="recbc", name=f"recbc{u}{half}")
        stg_o = tailp.tile([64, 2, 128], BF16, tag="stgo", name=f"stgo{u}{half}")

        def stage_avd(avh=avh, avd=avd):
            nc.scalar.copy(avd, avh)

        def stage_dendma(avd=avd, den_t=den_t):
            nc.sync.dma_start(out=den_t, in_=avd[D : D + 1, :])

        def stage_recip(den_t=den_t):
            nc.vector.reciprocal_approx_fast(out=den_t, in_=den_t)

        def stage_recdma(den_t=den_t, rec_d=rec_d):
            # bounce the reciprocals through DRAM: a DRAM source AP may be
            # partition-broadcast, and the in-loop gpsimd partition_broadcast
            # ISA op costs a ~6us GPSIMD microcode library swap against the
            # normalize tensor_tensors
            nc.sync.dma_start(out=rec_d.rearrange("o (t a) -> (o t) a", t=4), in_=den_t)

        def stage_bcast(rec_d=rec_d, rec_bc=rec_bc):
            nc.sync.dma_start(out=rec_bc, in_=rec_d.partition_broadcast(64))

        def stage_norm_even(avd=avd, rec_bc=rec_bc, h=h):
            av_r = avd[0:D].rearrange("d (rl a) -> d rl a", a=128)
            bc_r = rec_bc.rearrange("d (rl a) -> d rl a", a=128)
            # even rl (r = 4iq+rl even): partitions already correct (e%128 = d)
            nc.gpsimd.tensor_tensor(
                out=oT[0:64, 2 * iq : 2 * iq + 2, 128 * h : 128 * h + 128],
                in0=av_r[:, 0::2, :],
                in1=bc_r[:, 0::2, :],
                op=mybir.AluOpType.mult,
            )

        def stage_norm_odd(avd=avd, rec_bc=rec_bc, stg_o=stg_o):
            av_r = avd[0:D].rearrange("d (rl a) -> d rl a", a=128)
            bc_r = rec_bc.rearrange("d (rl a) -> d rl a", a=128)
            nc.gpsimd.tensor_tensor(
                out=stg_o,
                in0=av_r[:, 1::2, :],
                in1=bc_r[:, 1::2, :],
                op=mybir.AluOpType.mult,
            )

        def stage_stgdma(stg_o=stg_o, h=h):
            # odd rl: partition-shift DMA to oT[64:], on the idle sync queue
            nc.sync.dma_start(
                out=oT[64:128, 2 * iq : 2 * iq + 2, 128 * h : 128 * h + 128],
                in_=stg_o,
            )

        tail_stages.append(stage_avd)
        tail_stages.append(stage_dendma)
        tail_stages.append(stage_recip)
        tail_stages.append(stage_recdma)
        tail_stages.append(stage_bcast)
        tail_stages.append(stage_norm_even)
        tail_stages.append(stage_norm_odd)
        tail_stages.append(stage_stgdma)


_NC_CACHE = {}


def get_nc():
    if "nc" not in _NC_CACHE:
        _NC_CACHE["nc"] = build_nc()
    return _NC_CACHE["nc"]


def shard_inputs(q, k, v, wq, bq, wk, bk, wv, bv, wo, bo):
    """Build the 8 per-core input maps (host-side transposes/augments)."""

    import ml_dtypes

    bf16 = ml_dtypes.bfloat16
    f8 = ml_dtypes.float8_e4m3fn

    def aug_w(w, b):
        return np.concatenate(
            [np.ascontiguousarray(np.asarray(w, np.float32).T),
             np.asarray(b, np.float32)[None, :]],
            axis=0,
        ).astype(bf16)

    def w8_il(w):
        # [E, E] -> [4, 128, 2, E]: chunk k, partition j, o holds contract
        # row 256k + 2j + o of w.T (pre-interleaved DoubleRow layout)
        wt = np.ascontiguousarray(
            np.asarray(w, np.float32).T * np.float32(W8SCALE)
        ).astype(f8)
        return np.ascontiguousarray(wt.reshape(4, 128, 2, E))

    wq_a, wk_a = w8_il(wq), w8_il(wk)
    bq_a = (np.asarray(bq, np.float32) * np.float32(W8SCALE))[None, :].astype(bf16)
    bk_a = (np.asarray(bk, np.float32) * np.float32(W8SCALE))[None, :].astype(bf16)
    wv_a, wo_a = aug_w(wv, bv), aug_w(wo, bo)

    in_maps = []
    for c in range(N_CORES):
        b = c // 4
        r0 = 512 * (c % 4)
        sl = slice(r0, r0 + ROWS)

        def t_x(x, dt):
            return np.ascontiguousarray(np.asarray(x[b, sl, :], np.float32).T).astype(
                dt
            )

        def x8_il(x):
            return np.ascontiguousarray(t_x(x, f8).reshape(4, 128, 2, ROWS))

        in_maps.append(
            {
                "xq": x8_il(q),
                "xk": x8_il(k),
                "xv": t_x(v, bf16),
                "wq": wq_a,
                "wk": wk_a,
                "bq": bq_a,
                "bk": bk_a,
                "wv": wv_a,
                "wo": wo_a,
            }
        )
    return in_maps


def assemble_output(results):
    out = np.empty((B, S, E), np.float32)
    for c in range(N_CORES):
        b = c // 4
        r0 = 512 * (c % 4)
        out[b, r0 : r0 + ROWS, :] = results[c]["y"]
    return out


def kernel(q, k, v, wq, bq, wk, bk, wv, bv, wo, bo, **run_kwargs):
    nc = get_nc()
    in_maps = shard_inputs(q, k, v, wq, bq, wk, bk, wv, bv, wo, bo)
    res = run_bass_kernel_spmd(nc, in_maps, list(range(N_CORES)), **run_kwargs)
    out = assemble_output(res.results)
    if run_kwargs:
        return out, res
    return out
